# revision 3
# baseline (speedup 1.0000x reference)
"""BitSwarmLinear Trainium2 kernel.

Computation (reference):
    swarm_sum = population.sum(axis=2)          # (out, in)
    w_eff     = sign(swarm_sum), sign(0) -> +1  # (out, in), +-1
    y         = einsum("bsi,oi->bso", x, w_eff) # (4, 4096, out)

Distribution (8 NeuronCores, tensor-parallel on out_features):
    - population sharded on out_features: each core gets (256, 2048, 32),
      reduces + binarizes its slice and computes its 256 output columns.
    - x replicated to every core (staged pre-transposed as bf16 [in, tokens]
      so the contraction dim lands on SBUF partitions with line-rate DMA).
    - outputs gathered on the host along the feature dim.

Per-core device pipeline:
    1. Stream population slice (16 x 4MB DMAs), DVE-reduce the swarm axis,
       binarize via (s >= 0) * 2 - 1   (exact: sums are even ints, 0 -> +1).
    2. PE-transpose the 256x2048 sign matrix into W [in(part), out] bf16,
       resident in SBUF (1 MB).
    3. Stream x^T tiles [128 in, 512 tok] bf16; for each 128-token block
       accumulate 16 K-tile matmuls into PSUM [128 tok, 256 out] (fp32
       accumulate), copy to SBUF, store y tile contiguously.
"""

import os
import sys

import numpy as np

for _p in ("/root/.axon_site/_ro/trn_rl_repo", "/opt/trn_rl_repo"):
    if os.path.isdir(_p) and _p not in sys.path:
        sys.path.append(_p)

import ml_dtypes

import concourse.bass as bass  # noqa: F401  (AP helpers)
import concourse.mybir as mybir
import concourse.tile as tile
from concourse import bacc
from concourse.bass_utils import run_bass_kernel_spmd
from concourse.masks import make_identity

P = 128
IN_F = 2048
SWARM = 32
OUT_F = 2048
N_CORES = 8
OUT_C = OUT_F // N_CORES  # 256 out features per core
TOKENS = 4 * 4096

F32 = mybir.dt.float32
BF16 = mybir.dt.bfloat16

# population chunking along in_features for the swarm reduction
IC_CHUNK = 256
# token-block per x^T DMA / output store
TB = 512


def build_nc(tokens: int = TOKENS, out_c: int = OUT_C, in_f: int = IN_F):
    """Build the per-core Bass program (same program on all 8 cores)."""
    ko_tiles = in_f // P          # 16 K-tiles
    oc_groups = out_c // P        # 2 groups of 128 out rows
    ic_chunks = in_f // IC_CHUNK  # chunks per out-group for pop streaming
    tb_count = tokens // TB
    m_per_tb = TB // P

    nc = bacc.Bacc(
        "TRN2",
        target_bir_lowering=False,
        debug=False,
        enable_asserts=False,
        num_devices=N_CORES,
    )

    xT = nc.dram_tensor("xT", [in_f, tokens], BF16, kind="ExternalInput")
    pop = nc.dram_tensor("pop", [out_c, in_f, SWARM], F32, kind="ExternalInput")
    y = nc.dram_tensor("y", [tokens, out_c], F32, kind="ExternalOutput")

    xr = xT.ap().rearrange("(ko ki) t -> ki ko t", ki=P)        # [128, ko, T]
    pr = pop.ap()                                               # [out_c, in, 32]
    yr = y.ap().rearrange("(n m p) o -> n p m o", p=P, m=m_per_tb)

    with tile.TileContext(nc) as tc:
        with (
            tc.tile_pool(name="const", bufs=1) as const_pool,
            tc.tile_pool(name="pop", bufs=2) as pop_pool,
            tc.tile_pool(name="ss", bufs=oc_groups) as ss_pool,
            tc.tile_pool(name="wsb", bufs=1) as w_pool,
            tc.tile_pool(name="xt", bufs=3) as x_pool,
            tc.tile_pool(name="ystage", bufs=2) as y_pool,
            tc.tile_pool(name="psum_t", bufs=2, space="PSUM") as psum_t_pool,
            tc.tile_pool(name="psum_y", bufs=4, space="PSUM") as psum_y_pool,
        ):
            ident = const_pool.tile([P, P], F32)
            make_identity(nc, ident[:])

            # W in [in(part), ko, out] bf16 — matmul rhs tiles, SBUF-resident
            w_sb = w_pool.tile([P, ko_tiles, out_c], BF16)

            # ---- Stage 1: swarm reduce + binarize (natural [out, in] layout)
            sign_tiles = []
            for oc in range(oc_groups):
                ss = ss_pool.tile([P, in_f], F32, tag="ss")
                for ic in range(ic_chunks):
                    pt = pop_pool.tile([P, IC_CHUNK, SWARM], F32, tag="pop")
                    nc.sync.dma_start(
                        pt[:],
                        pr[
                            oc * P : (oc + 1) * P,
                            ic * IC_CHUNK : (ic + 1) * IC_CHUNK,
                            :,
                        ],
                    )
                    nc.vector.reduce_sum(
                        out=ss[:, ic * IC_CHUNK : (ic + 1) * IC_CHUNK],
                        in_=pt[:],
                        axis=mybir.AxisListType.X,
                    )
                # sign with sign(0) -> +1:  w = (ss >= 0) * 2 - 1
                nc.vector.tensor_scalar(
                    out=ss[:],
                    in0=ss[:],
                    scalar1=0.0,
                    scalar2=2.0,
                    op0=mybir.AluOpType.is_ge,
                    op1=mybir.AluOpType.mult,
                )
                nc.vector.tensor_scalar(
                    out=ss[:],
                    in0=ss[:],
                    scalar1=1.0,
                    scalar2=None,
                    op0=mybir.AluOpType.subtract,
                )
                sign_tiles.append(ss)

            # ---- Stage 2: PE-transpose sign matrix into W [in, out] bf16
            for oc in range(oc_groups):
                ss = sign_tiles[oc]
                for k in range(ko_tiles):
                    pt_ps = psum_t_pool.tile([P, P], F32, tag="tps")
                    nc.tensor.transpose(
                        pt_ps[:], ss[:, k * P : (k + 1) * P], ident[:]
                    )
                    nc.vector.tensor_copy(
                        out=w_sb[:, k, oc * P : (oc + 1) * P], in_=pt_ps[:]
                    )

            # ---- Stage 3: stream x^T, matmul, store y
            for tb in range(tb_count):
                xt = x_pool.tile([P, ko_tiles, TB], BF16, tag="xt")
                nc.sync.dma_start(xt[:], xr[:, :, tb * TB : (tb + 1) * TB])
                ystage = y_pool.tile([P, m_per_tb, out_c], F32, tag="ys")
                for m in range(m_per_tb):
                    ps = psum_y_pool.tile([P, out_c], F32, tag="yps")
                    for k in range(ko_tiles):
                        nc.tensor.matmul(
                            ps[:],
                            xt[:, k, m * P : (m + 1) * P],
                            w_sb[:, k, :],
                            start=(k == 0),
                            stop=(k == ko_tiles - 1),
                        )
                    nc.vector.tensor_copy(out=ystage[:, m, :], in_=ps[:])
                nc.sync.dma_start(yr[tb], ystage[:])

    nc.compile()  # bacc register allocation / DCE — required before codegen
    return nc


_NC_CACHE: dict = {}


def _get_nc(tokens=TOKENS, out_c=OUT_C, in_f=IN_F):
    key = (tokens, out_c, in_f)
    if key not in _NC_CACHE:
        _NC_CACHE[key] = build_nc(*key)
    return _NC_CACHE[key]


def prep_inputs(x: np.ndarray, population: np.ndarray):
    """Host-side sharding/staging: x -> replicated bf16 [in, tokens];
    population -> per-core out_features slices."""
    tokens = x.shape[0] * x.shape[1]
    in_f = x.shape[2]
    xT = np.ascontiguousarray(
        x.reshape(tokens, in_f).T.astype(ml_dtypes.bfloat16)
    )
    out_c = population.shape[0] // N_CORES
    in_maps = []
    for c in range(N_CORES):
        pop_c = np.ascontiguousarray(
            population[c * out_c : (c + 1) * out_c]
        ).astype(np.float32, copy=False)
        in_maps.append({"xT": xT, "pop": pop_c})
    return in_maps, tokens, out_c, in_f


def kernel(x: np.ndarray, population: np.ndarray):
    in_maps, tokens, out_c, in_f = prep_inputs(x, population)
    nc = _get_nc(tokens, out_c, in_f)
    res = run_bass_kernel_spmd(nc, in_maps, core_ids=list(range(N_CORES)))
    y_full = np.concatenate([r["y"] for r in res.results], axis=1)
    return y_full.reshape(x.shape[0], x.shape[1], population.shape[0])


# revision 6
# speedup vs baseline: 3.4220x; 3.4220x over previous
"""BitSwarmLinear Trainium2 kernel.

Computation (reference):
    swarm_sum = population.sum(axis=2)          # (out, in)
    w_eff     = sign(swarm_sum), sign(0) -> +1  # (out, in), +-1
    y         = einsum("bsi,oi->bso", x, w_eff) # (4, 4096, out)

Distribution (8 NeuronCores, tensor-parallel on out_features):
    - population sharded on out_features: each core gets (256, 2048, 32),
      reduces + binarizes its slice and computes its 256 output columns.
    - x replicated to every core (staged pre-transposed as bf16 [in, tokens]
      so the contraction dim lands on SBUF partitions with line-rate DMA).
    - outputs gathered on the host along the feature dim.

Per-core device pipeline:
    1. Stream population slice (16 x 4MB DMAs), DVE-reduce the swarm axis,
       binarize via (s >= 0) * 2 - 1   (exact: sums are even ints, 0 -> +1).
    2. PE-transpose the 256x2048 sign matrix into W [in(part), out] bf16,
       resident in SBUF (1 MB).
    3. Stream x^T tiles [128 in, 512 tok] bf16; for each 128-token block
       accumulate 16 K-tile matmuls into PSUM [128 tok, 256 out] (fp32
       accumulate), copy to SBUF, store y tile contiguously.
"""

import os
import sys

import numpy as np

for _p in ("/root/.axon_site/_ro/trn_rl_repo", "/opt/trn_rl_repo"):
    if os.path.isdir(_p) and _p not in sys.path:
        sys.path.append(_p)

import ml_dtypes

import concourse.bass as bass  # noqa: F401  (AP helpers)
import concourse.mybir as mybir
import concourse.tile as tile
from concourse import bacc
from concourse.bass_utils import run_bass_kernel_spmd
from concourse.masks import make_identity

P = 128
IN_F = 2048
SWARM = 32
OUT_F = 2048
N_CORES = 8
OUT_C = OUT_F // N_CORES  # 256 out features per core
TOKENS = 4 * 4096

F32 = mybir.dt.float32
BF16 = mybir.dt.bfloat16

# population chunking along in_features for the swarm reduction
IC_CHUNK = 256
# token-block per x^T DMA / output store
TB = 512


def build_nc(tokens: int = TOKENS, out_c: int = OUT_C, in_f: int = IN_F,
             reps: int = 1):
    """Build the per-core Bass program (same program on all 8 cores).

    reps>1 repeats the whole pipeline back-to-back (timing harness only)."""
    ko_tiles = in_f // P          # 16 K-tiles
    oc_groups = out_c // P        # 2 groups of 128 out rows
    ic_chunks = in_f // IC_CHUNK  # chunks per out-group for pop streaming
    tb_count = tokens // TB
    m_per_tb = TB // P

    nc = bacc.Bacc(
        "TRN2",
        target_bir_lowering=False,
        debug=False,
        enable_asserts=False,
        num_devices=N_CORES,
    )

    xT = nc.dram_tensor("xT", [in_f, tokens], BF16, kind="ExternalInput")
    pop = nc.dram_tensor("pop", [out_c, in_f, SWARM], F32, kind="ExternalInput")
    y = nc.dram_tensor("y", [tokens, out_c], F32, kind="ExternalOutput")

    xr = xT.ap().rearrange("(ko ki) t -> ki ko t", ki=P)        # [128, ko, T]
    pr = pop.ap()                                               # [out_c, in, 32]
    yr = y.ap().rearrange("(n m p) o -> n p m o", p=P, m=m_per_tb)

    with tile.TileContext(nc) as tc:
        with (
            tc.tile_pool(name="const", bufs=1) as const_pool,
            tc.tile_pool(name="pop", bufs=2) as pop_pool,
            tc.tile_pool(name="ss", bufs=oc_groups) as ss_pool,
            tc.tile_pool(name="wsb", bufs=1) as w_pool,
            tc.tile_pool(name="xt", bufs=3) as x_pool,
            tc.tile_pool(name="ystage", bufs=2) as y_pool,
            tc.tile_pool(name="psum_t", bufs=2, space="PSUM") as psum_t_pool,
            tc.tile_pool(name="psum_y", bufs=4, space="PSUM") as psum_y_pool,
        ):
            ident = const_pool.tile([P, P], F32)
            make_identity(nc, ident[:])

            for _rep in range(reps):
                _emit_body(
                    nc, tc, ident, w_pool, pop_pool, ss_pool, x_pool, y_pool,
                    psum_t_pool, psum_y_pool, pr, xr, yr,
                    oc_groups, ic_chunks, ko_tiles, tb_count, m_per_tb, out_c,
                    in_f,
                )

    nc.compile()  # bacc register allocation / DCE — required before codegen
    return nc


def _emit_body(nc, tc, ident, w_pool, pop_pool, ss_pool, x_pool, y_pool,
               psum_t_pool, psum_y_pool, pr, xr, yr,
               oc_groups, ic_chunks, ko_tiles, tb_count, m_per_tb, out_c,
               in_f):
    if True:
        if True:
            # W in [in(part), ko, out] bf16 — matmul rhs tiles, SBUF-resident
            w_sb = w_pool.tile([P, ko_tiles, out_c], BF16, tag="wsb")

            # ---- Stage 1: swarm reduce + binarize (natural [out, in] layout)
            sign_tiles = []
            for oc in range(oc_groups):
                ss = ss_pool.tile([P, in_f], F32, tag="ss")
                for ic in range(ic_chunks):
                    pt = pop_pool.tile([P, IC_CHUNK, SWARM], F32, tag="pop")
                    nc.sync.dma_start(
                        pt[:],
                        pr[
                            oc * P : (oc + 1) * P,
                            ic * IC_CHUNK : (ic + 1) * IC_CHUNK,
                            :,
                        ],
                    )
                    nc.vector.reduce_sum(
                        out=ss[:, ic * IC_CHUNK : (ic + 1) * IC_CHUNK],
                        in_=pt[:],
                        axis=mybir.AxisListType.X,
                    )
                # sign with sign(0) -> +1:  w = (ss >= 0) * 2 - 1
                nc.vector.tensor_scalar(
                    out=ss[:],
                    in0=ss[:],
                    scalar1=0.0,
                    scalar2=2.0,
                    op0=mybir.AluOpType.is_ge,
                    op1=mybir.AluOpType.mult,
                )
                nc.vector.tensor_scalar(
                    out=ss[:],
                    in0=ss[:],
                    scalar1=1.0,
                    scalar2=None,
                    op0=mybir.AluOpType.subtract,
                )
                sign_tiles.append(ss)

            # ---- Stage 2: PE-transpose sign matrix into W [in, out] bf16
            for oc in range(oc_groups):
                ss = sign_tiles[oc]
                for k in range(ko_tiles):
                    pt_ps = psum_t_pool.tile([P, P], F32, tag="tps")
                    nc.tensor.transpose(
                        pt_ps[:], ss[:, k * P : (k + 1) * P], ident[:]
                    )
                    nc.vector.tensor_copy(
                        out=w_sb[:, k, oc * P : (oc + 1) * P], in_=pt_ps[:]
                    )

            # ---- Stage 3: stream x^T, matmul, store y
            for tb in range(tb_count):
                xt = x_pool.tile([P, ko_tiles, TB], BF16, tag="xt")
                nc.sync.dma_start(xt[:], xr[:, :, tb * TB : (tb + 1) * TB])
                ystage = y_pool.tile([P, m_per_tb, out_c], F32, tag="ys")
                for m in range(m_per_tb):
                    ps = psum_y_pool.tile([P, out_c], F32, tag="yps")
                    for k in range(ko_tiles):
                        nc.tensor.matmul(
                            ps[:],
                            xt[:, k, m * P : (m + 1) * P],
                            w_sb[:, k, :],
                            start=(k == 0),
                            stop=(k == ko_tiles - 1),
                        )
                    nc.vector.tensor_copy(out=ystage[:, m, :], in_=ps[:])
                nc.sync.dma_start(yr[tb], ystage[:])


_NC_CACHE: dict = {}


def _get_nc(tokens=TOKENS, out_c=OUT_C, in_f=IN_F):
    key = (tokens, out_c, in_f)
    if key not in _NC_CACHE:
        _NC_CACHE[key] = build_nc(*key)
    return _NC_CACHE[key]


def prep_inputs(x: np.ndarray, population: np.ndarray):
    """Host-side sharding/staging: x -> replicated bf16 [in, tokens];
    population -> per-core out_features slices."""
    tokens = x.shape[0] * x.shape[1]
    in_f = x.shape[2]
    xT = np.ascontiguousarray(
        x.reshape(tokens, in_f).T.astype(ml_dtypes.bfloat16)
    )
    out_c = population.shape[0] // N_CORES
    in_maps = []
    for c in range(N_CORES):
        pop_c = np.ascontiguousarray(
            population[c * out_c : (c + 1) * out_c]
        ).astype(np.float32, copy=False)
        in_maps.append({"xT": xT, "pop": pop_c})
    return in_maps, tokens, out_c, in_f


def kernel(x: np.ndarray, population: np.ndarray):
    in_maps, tokens, out_c, in_f = prep_inputs(x, population)
    nc = _get_nc(tokens, out_c, in_f)
    res = run_bass_kernel_spmd(nc, in_maps, core_ids=list(range(N_CORES)))
    y_full = np.concatenate([r["y"] for r in res.results], axis=1)
    return y_full.reshape(x.shape[0], x.shape[1], population.shape[0])


# revision 12
# speedup vs baseline: 4.0232x; 1.1757x over previous
"""BitSwarmLinear Trainium2 kernel.

Computation (reference):
    swarm_sum = population.sum(axis=2)          # (out, in)
    w_eff     = sign(swarm_sum), sign(0) -> +1  # (out, in), +-1
    y         = einsum("bsi,oi->bso", x, w_eff) # (4, 4096, out)

Distribution (8 NeuronCores, tensor-parallel on out_features):
    - population sharded on out_features: each core gets (256, 2048, 32),
      reduces + binarizes its slice and computes its 256 output columns.
    - x replicated to every core (staged pre-transposed as bf16 [in, tokens]
      so the contraction dim lands on SBUF partitions with line-rate DMA).
    - outputs gathered on the host along the feature dim.

Per-core device pipeline (DMA-bound; ~151 MB/core HBM traffic):
    1. Stream population slice (16 x 4MB DMAs split over both HWDGE rings),
       DVE-reduce the swarm axis, binarize via (s >= 0) * 2 - 1 (exact:
       sums are even ints, 0 -> +1).  in_features-major order so W K-tiles
       complete early.
    2. PE-transpose the sign matrix into W [in(part), out] bf16, SBUF
       resident (1 MB), interleaved with the streaming.
    3. Stream x^T tiles [128 in, 16 ko, 512 tok] bf16 (deep prefetch);
       per 128-token block accumulate 16 K-tile matmuls into PSUM
       [128 tok, 256 out] (fp32 accumulate), copy+round to bf16, store.
       Stores ride the scalar-engine HWDGE ring so loads never queue
       behind them.  Host converts y back to f32.
"""

import os
import sys

import numpy as np

for _p in ("/root/.axon_site/_ro/trn_rl_repo", "/opt/trn_rl_repo"):
    if os.path.isdir(_p) and _p not in sys.path:
        sys.path.append(_p)

import ml_dtypes

import concourse.bass as bass  # noqa: F401  (AP helpers)
import concourse.mybir as mybir
import concourse.tile as tile
from concourse import bacc
from concourse.bass_utils import run_bass_kernel_spmd
from concourse.masks import make_identity

P = 128
IN_F = 2048
SWARM = 32
OUT_F = 2048
N_CORES = 8
OUT_C = OUT_F // N_CORES  # 256 out features per core
TOKENS = 4 * 4096

F32 = mybir.dt.float32
BF16 = mybir.dt.bfloat16
I8 = mybir.dt.int8

# population chunking along in_features for the swarm reduction
IC_CHUNK = 256
# token-block per x^T DMA / output store
TB = 1024
# x^T prefetch depth (SBUF: 32KB/partition each at TB=1024)
XT_BUFS = 4


def build_nc(tokens: int = TOKENS, out_c: int = OUT_C, in_f: int = IN_F,
             reps: int = 1):
    """Build the per-core Bass program (same program on all 8 cores).

    reps>1 repeats the whole pipeline back-to-back (timing harness only)."""
    ko_tiles = in_f // P          # 16 K-tiles
    oc_groups = out_c // P        # 2 groups of 128 out rows
    ic_chunks = in_f // IC_CHUNK  # chunks per out-group for pop streaming
    tb_count = tokens // TB
    m_per_tb = TB // P

    nc = bacc.Bacc(
        "TRN2",
        target_bir_lowering=False,
        debug=False,
        enable_asserts=False,
        num_devices=N_CORES,
    )

    xT = nc.dram_tensor("xT", [in_f, tokens], BF16, kind="ExternalInput")
    # population values are exactly +-1.0, so int8 staging is lossless and
    # cuts the dominant input stream 4x.
    pop = nc.dram_tensor("pop", [out_c, in_f, SWARM], I8, kind="ExternalInput")
    y = nc.dram_tensor("y", [tokens, out_c], BF16, kind="ExternalOutput")

    xr = xT.ap().rearrange("(ko ki) t -> ki ko t", ki=P)        # [128, ko, T]
    pr = pop.ap()                                               # [out_c, in, 32]
    yr = y.ap().rearrange("(n m p) o -> n p m o", p=P, m=m_per_tb)

    with tile.TileContext(nc) as tc:
        with (
            tc.tile_pool(name="const", bufs=1) as const_pool,
            tc.tile_pool(name="pop", bufs=2) as pop_pool,
            tc.tile_pool(name="ss", bufs=oc_groups) as ss_pool,
            tc.tile_pool(name="wsb", bufs=1) as w_pool,
            tc.tile_pool(name="xt", bufs=XT_BUFS) as x_pool,
            tc.tile_pool(name="ystage", bufs=2) as y_pool,
            tc.tile_pool(name="psum_t", bufs=2, space="PSUM") as psum_t_pool,
            tc.tile_pool(name="psum_y", bufs=4, space="PSUM") as psum_y_pool,
        ):
            ident = const_pool.tile([P, P], F32)
            make_identity(nc, ident[:])

            for _rep in range(reps):
                _emit_body(
                    nc, ident, w_pool, pop_pool, ss_pool, x_pool, y_pool,
                    psum_t_pool, psum_y_pool, pr, xr, yr,
                    oc_groups, ic_chunks, ko_tiles, tb_count, m_per_tb, out_c,
                    in_f,
                )

    nc.compile()  # bacc register allocation / DCE — required before codegen
    return nc


def _emit_body(nc, ident, w_pool, pop_pool, ss_pool, x_pool, y_pool,
               psum_t_pool, psum_y_pool, pr, xr, yr,
               oc_groups, ic_chunks, ko_tiles, tb_count, m_per_tb, out_c,
               in_f):
    # W in [in(part), ko, out] bf16 — matmul rhs tiles, SBUF-resident
    w_sb = w_pool.tile([P, ko_tiles, out_c], BF16, tag="wsb")
    ss_tiles = [
        ss_pool.tile([P, in_f], F32, tag="ss", name=f"ss{oc}")
        for oc in range(oc_groups)
    ]
    ko_per_ic = IC_CHUNK // P  # K-tiles completed per ic chunk

    # ---- Stage 1+2: swarm reduce + binarize + PE-transpose into W,
    # in_features-major so W K-tiles complete as the stream advances.
    for ic in range(ic_chunks):
        isl = slice(ic * IC_CHUNK, (ic + 1) * IC_CHUNK)
        for oc in range(oc_groups):
            pt = pop_pool.tile([P, IC_CHUNK, SWARM], I8, tag="pop")
            # alternate the two HWDGE rings to overlap descriptor setup
            eng = nc.sync if (ic * oc_groups + oc) % 2 == 0 else nc.scalar
            eng.dma_start(pt[:], pr[oc * P : (oc + 1) * P, isl, :])
            ss = ss_tiles[oc]
            nc.vector.reduce_sum(
                out=ss[:, isl], in_=pt[:], axis=mybir.AxisListType.X
            )
            # sign with sign(0) -> +1:  w = (ss >= 0) * 2 - 1
            nc.vector.tensor_scalar(
                out=ss[:, isl], in0=ss[:, isl], scalar1=0.0, scalar2=2.0,
                op0=mybir.AluOpType.is_ge, op1=mybir.AluOpType.mult,
            )
            nc.vector.tensor_scalar(
                out=ss[:, isl], in0=ss[:, isl], scalar1=1.0, scalar2=None,
                op0=mybir.AluOpType.subtract,
            )
        for kk in range(ko_per_ic):
            k = ic * ko_per_ic + kk
            for oc in range(oc_groups):
                pt_ps = psum_t_pool.tile([P, P], F32, tag="tps")
                nc.tensor.transpose(
                    pt_ps[:], ss_tiles[oc][:, k * P : (k + 1) * P], ident[:]
                )
                nc.vector.tensor_copy(
                    out=w_sb[:, k, oc * P : (oc + 1) * P], in_=pt_ps[:]
                )

    # ---- Stage 3: stream x^T, matmul, store y (bf16)
    for tb in range(tb_count):
        xt = x_pool.tile([P, ko_tiles, TB], BF16, tag="xt")
        nc.sync.dma_start(xt[:], xr[:, :, tb * TB : (tb + 1) * TB])
        ystage = y_pool.tile([P, m_per_tb, out_c], BF16, tag="ys")
        for m in range(m_per_tb):
            ps = psum_y_pool.tile([P, out_c], F32, tag="yps")
            for k in range(ko_tiles):
                nc.tensor.matmul(
                    ps[:],
                    xt[:, k, m * P : (m + 1) * P],
                    w_sb[:, k, :],
                    start=(k == 0),
                    stop=(k == ko_tiles - 1),
                )
            nc.vector.tensor_copy(out=ystage[:, m, :], in_=ps[:])
        # stores ride the ACT HWDGE ring; loads own the SP ring
        nc.scalar.dma_start(yr[tb], ystage[:])


_NC_CACHE: dict = {}


def _get_nc(tokens=TOKENS, out_c=OUT_C, in_f=IN_F):
    key = (tokens, out_c, in_f)
    if key not in _NC_CACHE:
        _NC_CACHE[key] = build_nc(*key)
    return _NC_CACHE[key]


def prep_inputs(x: np.ndarray, population: np.ndarray):
    """Host-side sharding/staging: x -> replicated bf16 [in, tokens];
    population -> per-core out_features slices."""
    tokens = x.shape[0] * x.shape[1]
    in_f = x.shape[2]
    xT = np.ascontiguousarray(
        x.reshape(tokens, in_f).T.astype(ml_dtypes.bfloat16)
    )
    out_c = population.shape[0] // N_CORES
    in_maps = []
    for c in range(N_CORES):
        # +-1.0 float -> +-1 int8 is exact
        pop_c = np.ascontiguousarray(
            population[c * out_c : (c + 1) * out_c].astype(np.int8)
        )
        in_maps.append({"xT": xT, "pop": pop_c})
    return in_maps, tokens, out_c, in_f


def kernel(x: np.ndarray, population: np.ndarray):
    in_maps, tokens, out_c, in_f = prep_inputs(x, population)
    nc = _get_nc(tokens, out_c, in_f)
    res = run_bass_kernel_spmd(nc, in_maps, core_ids=list(range(N_CORES)))
    y_full = np.concatenate(
        [r["y"].astype(np.float32) for r in res.results], axis=1
    )
    return y_full.reshape(x.shape[0], x.shape[1], population.shape[0])


# revision 13
# speedup vs baseline: 4.5604x; 1.1335x over previous
"""BitSwarmLinear Trainium2 kernel.

Computation (reference):
    swarm_sum = population.sum(axis=2)          # (out, in)
    w_eff     = sign(swarm_sum), sign(0) -> +1  # (out, in), +-1
    y         = einsum("bsi,oi->bso", x, w_eff) # (4, 4096, out)

Distribution (8 NeuronCores, tensor-parallel on out_features):
    - population sharded on out_features: each core gets its 256 rows,
      reduces + binarizes them and computes its 256 output columns.
    - x replicated to every core, staged pre-transposed/tiled as bf16 so the
      contraction dim lands on SBUF partitions with fully-contiguous DMA.
    - outputs gathered on the host along the feature dim.

Host staging (lossless / layout-only):
    - population is exactly +-1.0 -> int8, rearranged swarm-major
      [32, out_c, in]: cuts the dominant input stream 4x and lets the DMA
      engines' inline CCE ALU do the swarm reduction during transfer.
    - x -> bf16 x^T, tiled [tb, 128 ki, 16 ko, TB tok] so every DMA line is
      a 32KB contiguous run (line-rate HBM).
    - y comes back bf16 tile-major; host restores [b, s, out] f32.

Per-core device pipeline:
    1. Four parallel SWDGE accumulate chains (8 DMAs each, CCE int8 add)
       reduce the swarm axis while transferring; DVE merges 4 partials,
       binarizes via (s >= 0) * 2 - 1 (exact: sums are even ints, 0 -> +1).
    2. PE-transpose the sign matrix into W [in(part), out] bf16 (SBUF
       resident, 1 MB).
    3. Stream x^T tiles (4MB contiguous DMAs, deep prefetch); per 128-token
       block accumulate 16 K-tile matmuls into PSUM [128 tok, 256 out]
       (fp32), round to bf16, store on the scalar HWDGE ring.
"""

import os
import sys

import numpy as np

for _p in ("/root/.axon_site/_ro/trn_rl_repo", "/opt/trn_rl_repo"):
    if os.path.isdir(_p) and _p not in sys.path:
        sys.path.append(_p)

import ml_dtypes

import concourse.bass as bass  # noqa: F401  (AP helpers)
import concourse.mybir as mybir
import concourse.tile as tile
from concourse import bacc
from concourse.bass_utils import run_bass_kernel_spmd
from concourse.masks import make_identity

P = 128
IN_F = 2048
SWARM = 32
OUT_F = 2048
N_CORES = 8
OUT_C = OUT_F // N_CORES  # 256 out features per core
TOKENS = 4 * 4096

F32 = mybir.dt.float32
BF16 = mybir.dt.bfloat16
I8 = mybir.dt.int8

# token-block per x^T DMA / output store
TB = 1024
# x^T prefetch depth (SBUF: 32KB/partition each at TB=1024)
XT_BUFS = 4
# parallel CCE-accumulate chains for the swarm reduction
ACC_CHAINS = 4


def build_nc(tokens: int = TOKENS, out_c: int = OUT_C, in_f: int = IN_F,
             reps: int = 1):
    """Build the per-core Bass program (same program on all 8 cores).

    reps>1 repeats the whole pipeline back-to-back (timing harness only)."""
    ko_tiles = in_f // P          # 16 K-tiles
    oc_groups = out_c // P        # 2 groups of 128 out rows
    tb_count = tokens // TB
    m_per_tb = TB // P

    nc = bacc.Bacc(
        "TRN2",
        target_bir_lowering=False,
        debug=False,
        enable_asserts=False,
        num_devices=N_CORES,
    )

    xT = nc.dram_tensor("xT", [tb_count, P, ko_tiles, TB], BF16,
                        kind="ExternalInput")
    pop = nc.dram_tensor("pop", [SWARM, out_c, in_f], I8,
                         kind="ExternalInput")
    y = nc.dram_tensor("y", [tb_count, P, m_per_tb, out_c], BF16,
                       kind="ExternalOutput")

    xr = xT.ap()                                              # [tb,128,ko,TB]
    pr = pop.ap().rearrange("s (g p) i -> s p g i", p=P)      # [32,128,oc,in]
    yr = y.ap()                                               # [tb,128,m,oc*P]

    with tile.TileContext(nc) as tc:
        with (
            tc.tile_pool(name="const", bufs=1) as const_pool,
            tc.tile_pool(name="acc", bufs=ACC_CHAINS) as acc_pool,
            tc.tile_pool(name="sgn", bufs=oc_groups) as sgn_pool,
            tc.tile_pool(name="wsb", bufs=1) as w_pool,
            tc.tile_pool(name="xt", bufs=XT_BUFS) as x_pool,
            tc.tile_pool(name="ystage", bufs=2) as y_pool,
            tc.tile_pool(name="psum_t", bufs=2, space="PSUM") as psum_t_pool,
            tc.tile_pool(name="psum_y", bufs=4, space="PSUM") as psum_y_pool,
        ):
            ident = const_pool.tile([P, P], F32)
            make_identity(nc, ident[:])

            for _rep in range(reps):
                _emit_body(
                    nc, ident, w_pool, acc_pool, sgn_pool, x_pool, y_pool,
                    psum_t_pool, psum_y_pool, pr, xr, yr,
                    oc_groups, ko_tiles, tb_count, m_per_tb, out_c, in_f,
                )

    nc.compile()  # bacc register allocation / DCE — required before codegen
    return nc


def _emit_body(nc, ident, w_pool, acc_pool, sgn_pool, x_pool, y_pool,
               psum_t_pool, psum_y_pool, pr, xr, yr,
               oc_groups, ko_tiles, tb_count, m_per_tb, out_c, in_f):
    # W in [in(part), ko, out] bf16 — matmul rhs tiles, SBUF-resident
    w_sb = w_pool.tile([P, ko_tiles, out_c], BF16, tag="wsb")

    # ---- Stage 1: swarm reduction in the DMA engines (CCE int8 add).
    # 4 chains of 8 accumulating transfers, interleaved so they run in
    # parallel; sums stay within int8 (|sum| <= 32).
    s_per_chain = SWARM // ACC_CHAINS
    accs = [
        acc_pool.tile([P, oc_groups, in_f], I8, tag="acc", name=f"acc{g}")
        for g in range(ACC_CHAINS)
    ]
    for j in range(s_per_chain):
        for g in range(ACC_CHAINS):
            s = g * s_per_chain + j
            nc.gpsimd.dma_start(
                accs[g][:],
                pr[s],
                accum_op=(
                    mybir.AluOpType.bypass if j == 0 else mybir.AluOpType.add
                ),
            )
    # merge the 4 partials (int8, max |sum| 32)
    nc.vector.tensor_add(accs[0][:], accs[0][:], accs[1][:])
    nc.vector.tensor_add(accs[2][:], accs[2][:], accs[3][:])
    nc.vector.tensor_add(accs[0][:], accs[0][:], accs[2][:])

    # ---- Stage 2: binarize + PE-transpose into W [in, out] bf16
    for oc in range(oc_groups):
        sgn = sgn_pool.tile([P, in_f], F32, tag="sgn", name=f"sgn{oc}")
        # sign with sign(0) -> +1:  w = (acc >= 0) * 2 - 1
        nc.vector.tensor_scalar(
            out=sgn[:], in0=accs[0][:, oc, :], scalar1=0.0, scalar2=2.0,
            op0=mybir.AluOpType.is_ge, op1=mybir.AluOpType.mult,
        )
        nc.vector.tensor_scalar(
            out=sgn[:], in0=sgn[:], scalar1=1.0, scalar2=None,
            op0=mybir.AluOpType.subtract,
        )
        for k in range(ko_tiles):
            pt_ps = psum_t_pool.tile([P, P], F32, tag="tps")
            nc.tensor.transpose(
                pt_ps[:], sgn[:, k * P : (k + 1) * P], ident[:]
            )
            nc.vector.tensor_copy(
                out=w_sb[:, k, oc * P : (oc + 1) * P], in_=pt_ps[:]
            )

    # ---- Stage 3: stream x^T, matmul, store y (bf16)
    for tb in range(tb_count):
        xt = x_pool.tile([P, ko_tiles, TB], BF16, tag="xt")
        nc.sync.dma_start(xt[:], xr[tb])
        ystage = y_pool.tile([P, m_per_tb, out_c], BF16, tag="ys")
        for m in range(m_per_tb):
            ps = psum_y_pool.tile([P, out_c], F32, tag="yps")
            for k in range(ko_tiles):
                nc.tensor.matmul(
                    ps[:],
                    xt[:, k, m * P : (m + 1) * P],
                    w_sb[:, k, :],
                    start=(k == 0),
                    stop=(k == ko_tiles - 1),
                )
            nc.vector.tensor_copy(out=ystage[:, m, :], in_=ps[:])
        # stores ride the ACT HWDGE ring; loads own the SP ring
        nc.scalar.dma_start(yr[tb], ystage[:])


_NC_CACHE: dict = {}


def _get_nc(tokens=TOKENS, out_c=OUT_C, in_f=IN_F):
    key = (tokens, out_c, in_f)
    if key not in _NC_CACHE:
        _NC_CACHE[key] = build_nc(*key)
    return _NC_CACHE[key]


def stage_x(x: np.ndarray, tokens: int, in_f: int):
    """x [b, s, in] f32 -> tiled bf16 [tb, 128 ki, ko, TB] of x^T."""
    xb = np.ascontiguousarray(
        x.reshape(tokens, in_f).T
    ).astype(ml_dtypes.bfloat16)  # [in, tokens]
    ko = in_f // P
    tb = tokens // TB
    # (ko ki) (tb t) -> tb ki ko t
    return np.ascontiguousarray(
        xb.reshape(ko, P, tb, TB).transpose(2, 1, 0, 3)
    )


def stage_pop_slice(pop_c: np.ndarray):
    """pop slice [out_c, in, 32] (+-1.0 f32) -> swarm-major int8
    [32, out_c, in]. Exact: +-1.0 -> +-1."""
    return np.ascontiguousarray(
        pop_c.astype(np.int8).transpose(2, 0, 1)
    )


def unstage_y(y_dev: np.ndarray, tokens: int, out_c: int):
    """y [tb, 128 p, m, out_c] bf16 -> [tokens, out_c] f32
    (token = tb*TB + m*128 + p)."""
    return (
        y_dev.astype(np.float32)
        .transpose(0, 2, 1, 3)
        .reshape(tokens, out_c)
    )


def prep_inputs(x: np.ndarray, population: np.ndarray):
    tokens = x.shape[0] * x.shape[1]
    in_f = x.shape[2]
    xT = stage_x(x, tokens, in_f)
    out_c = population.shape[0] // N_CORES
    in_maps = []
    for c in range(N_CORES):
        pop_c = stage_pop_slice(population[c * out_c : (c + 1) * out_c])
        in_maps.append({"xT": xT, "pop": pop_c})
    return in_maps, tokens, out_c, in_f


def kernel(x: np.ndarray, population: np.ndarray):
    in_maps, tokens, out_c, in_f = prep_inputs(x, population)
    nc = _get_nc(tokens, out_c, in_f)
    res = run_bass_kernel_spmd(nc, in_maps, core_ids=list(range(N_CORES)))
    y_full = np.concatenate(
        [unstage_y(r["y"], tokens, out_c) for r in res.results], axis=1
    )
    return y_full.reshape(x.shape[0], x.shape[1], population.shape[0])


# revision 17
# speedup vs baseline: 4.5829x; 1.0049x over previous
"""BitSwarmLinear Trainium2 kernel.

Computation (reference):
    swarm_sum = population.sum(axis=2)          # (out, in)
    w_eff     = sign(swarm_sum), sign(0) -> +1  # (out, in), +-1
    y         = einsum("bsi,oi->bso", x, w_eff) # (4, 4096, out)

Distribution (8 NeuronCores, tensor-parallel on out_features):
    - population sharded on out_features: each core gets its 256 rows,
      reduces + binarizes them and computes its 256 output columns.
    - x replicated to every core, staged pre-transposed/tiled as bf16 so the
      contraction dim lands on SBUF partitions with fully-contiguous DMA.
    - outputs gathered on the host along the feature dim.

Host staging (lossless / layout-only):
    - population is exactly +-1.0 -> int8, rearranged swarm-major
      [32, out_c, in]: cuts the dominant input stream 4x and lets the DMA
      engines' inline CCE ALU do the swarm reduction during transfer.
    - x -> bf16 x^T, tiled [tb, 128 ki, 16 ko, TB tok] so every DMA line is
      a 32KB contiguous run (line-rate HBM).
    - y comes back bf16 tile-major; host restores [b, s, out] f32.

Per-core device pipeline:
    1. Four parallel SWDGE accumulate chains (8 DMAs each, CCE int8 add)
       reduce the swarm axis while transferring; DVE merges 4 partials,
       binarizes via (s >= 0) * 2 - 1 (exact: sums are even ints, 0 -> +1).
    2. PE-transpose the sign matrix into W [in(part), out] bf16 (SBUF
       resident, 1 MB).
    3. Stream x^T tiles (4MB contiguous DMAs, deep prefetch); per 128-token
       block accumulate 16 K-tile matmuls into PSUM [128 tok, 256 out]
       (fp32), round to bf16, store on the scalar HWDGE ring.
"""

import os
import sys

import numpy as np

for _p in ("/root/.axon_site/_ro/trn_rl_repo", "/opt/trn_rl_repo"):
    if os.path.isdir(_p) and _p not in sys.path:
        sys.path.append(_p)

import ml_dtypes

# bass_utils' axon trace path imports antenv.axon_hooks, which this image
# lacks. Provide it (backed by the ctypes NTFF hook) so running with
# BASS_TRACE=1 works instead of crashing on the import.
try:
    import antenv.axon_hooks  # noqa: F401
except ImportError:
    try:
        import types as _types

        from trn_agent_boot.trn_boot import _ntff_profile_via_ctypes

        _hooks = _types.ModuleType("antenv.axon_hooks")
        _ntff_hook = _ntff_profile_via_ctypes("/opt/axon/libaxon_pjrt.so")
        _hooks.get_axon_ntff_profile_hook = lambda: _ntff_hook
        _hooks.set_axon_ntff_profile_hook = lambda h: None
        sys.modules["antenv.axon_hooks"] = _hooks
    except Exception:
        pass

import concourse.bass as bass  # noqa: F401  (AP helpers)
import concourse.mybir as mybir
import concourse.tile as tile
from concourse import bacc
from concourse.bass_utils import run_bass_kernel_spmd
from concourse.masks import make_identity

P = 128
IN_F = 2048
SWARM = 32
OUT_F = 2048
N_CORES = 8
OUT_C = OUT_F // N_CORES  # 256 out features per core
TOKENS = 4 * 4096

F32 = mybir.dt.float32
BF16 = mybir.dt.bfloat16
I8 = mybir.dt.int8

# token-block per x^T DMA / output store
TB = 1024
# x^T prefetch depth (SBUF: 32KB/partition each at TB=1024)
XT_BUFS = 4
# parallel CCE-accumulate chains for the swarm reduction.  8 chains of 4
# keep the GpSimd SWDGE queue free of head-of-line blocking: by the time a
# chain's next element reaches the queue head, its predecessor's completion
# (~5us transfer+receipt) has already landed.
ACC_CHAINS = 8


def build_nc(tokens: int = TOKENS, out_c: int = OUT_C, in_f: int = IN_F,
             reps: int = 1):
    """Build the per-core Bass program (same program on all 8 cores).

    reps>1 repeats the whole pipeline back-to-back (timing harness only)."""
    ko_tiles = in_f // P          # 16 K-tiles
    oc_groups = out_c // P        # 2 groups of 128 out rows
    tb_count = tokens // TB
    m_per_tb = TB // P

    nc = bacc.Bacc(
        "TRN2",
        target_bir_lowering=False,
        debug=False,
        enable_asserts=False,
        num_devices=N_CORES,
    )

    xT = nc.dram_tensor("xT", [tb_count, P, ko_tiles, TB], BF16,
                        kind="ExternalInput")
    pop = nc.dram_tensor("pop", [SWARM, out_c, in_f], I8,
                         kind="ExternalInput")
    y = nc.dram_tensor("y", [tb_count, P, m_per_tb, out_c], BF16,
                       kind="ExternalOutput")

    xr = xT.ap()                                              # [tb,128,ko,TB]
    pr = pop.ap().rearrange("s (g p) i -> s p g i", p=P)      # [32,128,oc,in]
    yr = y.ap()                                               # [tb,128,m,oc*P]

    with tile.TileContext(nc) as tc:
        with (
            tc.tile_pool(name="const", bufs=1) as const_pool,
            tc.tile_pool(name="acc", bufs=ACC_CHAINS) as acc_pool,
            tc.tile_pool(name="sgn", bufs=oc_groups) as sgn_pool,
            tc.tile_pool(name="wsb", bufs=1) as w_pool,
            tc.tile_pool(name="xt", bufs=XT_BUFS) as x_pool,
            tc.tile_pool(name="ystage", bufs=2) as y_pool,
            tc.tile_pool(name="psum_t", bufs=2, space="PSUM") as psum_t_pool,
            tc.tile_pool(name="psum_y", bufs=4, space="PSUM") as psum_y_pool,
        ):
            ident = const_pool.tile([P, P], F32)
            make_identity(nc, ident[:])

            for _rep in range(reps):
                _emit_body(
                    nc, ident, w_pool, acc_pool, sgn_pool, x_pool, y_pool,
                    psum_t_pool, psum_y_pool, pr, xr, yr,
                    oc_groups, ko_tiles, tb_count, m_per_tb, out_c, in_f,
                )

    nc.compile()  # bacc register allocation / DCE — required before codegen
    return nc


def _emit_body(nc, ident, w_pool, acc_pool, sgn_pool, x_pool, y_pool,
               psum_t_pool, psum_y_pool, pr, xr, yr,
               oc_groups, ko_tiles, tb_count, m_per_tb, out_c, in_f):
    # W in [in(part), ko, out] bf16 — matmul rhs tiles, SBUF-resident
    w_sb = w_pool.tile([P, ko_tiles, out_c], BF16, tag="wsb")

    # ---- Stage 1: swarm reduction in the DMA engines (CCE int8 add).
    # 4 chains of 8 accumulating transfers, interleaved so they run in
    # parallel; sums stay within int8 (|sum| <= 32).
    s_per_chain = SWARM // ACC_CHAINS
    accs = [
        acc_pool.tile([P, oc_groups, in_f], I8, tag="acc", name=f"acc{g}")
        for g in range(ACC_CHAINS)
    ]
    for j in range(s_per_chain):
        for g in range(ACC_CHAINS):
            s = g * s_per_chain + j
            nc.gpsimd.dma_start(
                accs[g][:],
                pr[s],
                accum_op=(
                    mybir.AluOpType.bypass if j == 0 else mybir.AluOpType.add
                ),
            )
    # tree-merge the partials on DVE; int8 sums stay within +-32
    stride = ACC_CHAINS // 2
    while stride >= 1:
        for g in range(stride):
            nc.vector.tensor_add(accs[g][:], accs[g][:], accs[g + stride][:])
        stride //= 2

    # ---- Stage 2: binarize + PE-transpose into W [in, out] bf16
    for oc in range(oc_groups):
        sgn = sgn_pool.tile([P, in_f], F32, tag="sgn", name=f"sgn{oc}")
        # sign with sign(0) -> +1:  w = (acc >= 0) * 2 - 1
        nc.vector.tensor_scalar(
            out=sgn[:], in0=accs[0][:, oc, :], scalar1=0.0, scalar2=2.0,
            op0=mybir.AluOpType.is_ge, op1=mybir.AluOpType.mult,
        )
        nc.vector.tensor_scalar(
            out=sgn[:], in0=sgn[:], scalar1=1.0, scalar2=None,
            op0=mybir.AluOpType.subtract,
        )
        for k in range(ko_tiles):
            pt_ps = psum_t_pool.tile([P, P], F32, tag="tps")
            nc.tensor.transpose(
                pt_ps[:], sgn[:, k * P : (k + 1) * P], ident[:]
            )
            nc.vector.tensor_copy(
                out=w_sb[:, k, oc * P : (oc + 1) * P], in_=pt_ps[:]
            )

    # ---- Stage 3: stream x^T, matmul, store y (bf16)
    for tb in range(tb_count):
        xt = x_pool.tile([P, ko_tiles, TB], BF16, tag="xt")
        nc.sync.dma_start(xt[:], xr[tb])
        ystage = y_pool.tile([P, m_per_tb, out_c], BF16, tag="ys")
        for m in range(m_per_tb):
            ps = psum_y_pool.tile([P, out_c], F32, tag="yps")
            for k in range(ko_tiles):
                nc.tensor.matmul(
                    ps[:],
                    xt[:, k, m * P : (m + 1) * P],
                    w_sb[:, k, :],
                    start=(k == 0),
                    stop=(k == ko_tiles - 1),
                )
            nc.vector.tensor_copy(out=ystage[:, m, :], in_=ps[:])
        # stores ride the ACT HWDGE ring; loads own the SP ring
        nc.scalar.dma_start(yr[tb], ystage[:])


_NC_CACHE: dict = {}


def _get_nc(tokens=TOKENS, out_c=OUT_C, in_f=IN_F):
    key = (tokens, out_c, in_f)
    if key not in _NC_CACHE:
        _NC_CACHE[key] = build_nc(*key)
    return _NC_CACHE[key]


def stage_x(x: np.ndarray, tokens: int, in_f: int):
    """x [b, s, in] f32 -> tiled bf16 [tb, 128 ki, ko, TB] of x^T."""
    xb = np.ascontiguousarray(
        x.reshape(tokens, in_f).T
    ).astype(ml_dtypes.bfloat16)  # [in, tokens]
    ko = in_f // P
    tb = tokens // TB
    # (ko ki) (tb t) -> tb ki ko t
    return np.ascontiguousarray(
        xb.reshape(ko, P, tb, TB).transpose(2, 1, 0, 3)
    )


def stage_pop_slice(pop_c: np.ndarray):
    """pop slice [out_c, in, 32] (+-1.0 f32) -> swarm-major int8
    [32, out_c, in]. Exact: +-1.0 -> +-1."""
    return np.ascontiguousarray(
        pop_c.astype(np.int8).transpose(2, 0, 1)
    )


def unstage_y(y_dev: np.ndarray, tokens: int, out_c: int):
    """y [tb, 128 p, m, out_c] bf16 -> [tokens, out_c] f32
    (token = tb*TB + m*128 + p)."""
    return (
        y_dev.astype(np.float32)
        .transpose(0, 2, 1, 3)
        .reshape(tokens, out_c)
    )


def prep_inputs(x: np.ndarray, population: np.ndarray):
    tokens = x.shape[0] * x.shape[1]
    in_f = x.shape[2]
    xT = stage_x(x, tokens, in_f)
    out_c = population.shape[0] // N_CORES
    in_maps = []
    for c in range(N_CORES):
        pop_c = stage_pop_slice(population[c * out_c : (c + 1) * out_c])
        in_maps.append({"xT": xT, "pop": pop_c})
    return in_maps, tokens, out_c, in_f


def kernel(x: np.ndarray, population: np.ndarray):
    in_maps, tokens, out_c, in_f = prep_inputs(x, population)
    nc = _get_nc(tokens, out_c, in_f)
    res = run_bass_kernel_spmd(nc, in_maps, core_ids=list(range(N_CORES)))
    y_full = np.concatenate(
        [unstage_y(r["y"], tokens, out_c) for r in res.results], axis=1
    )
    return y_full.reshape(x.shape[0], x.shape[1], population.shape[0])
